# revision 11
# baseline (speedup 1.0000x reference)
"""Trainium2 Bass kernel for nn_EdgeUpdate (gnn_message_passing).

reference math:
    atom_scalars = atom_features @ W_lin                       # [N, H]
    edge_in = concat([s[dst], s[src], edge_features], -1)      # [E, 3H]
    h = relu(edge_in @ W1 + b1); h = relu(h @ W2 + b2); h = h @ W3 + b3
    out = layernorm(edge_features + h) * gamma + beta          # [E, H]

Strategy: pure data-parallel over E across 8 cores (64000 edges each).
Per core:
  - build the full atom-scalar table on-chip ([H=128 partitions, N] fp32 in
    SBUF, 128KB/partition) from a host-transposed bf16 copy of atom_features
  - gather dst/src scalar columns per edge with gpsimd ap_gather (T-layout:
    features on partitions, edges on the free dim -> directly usable as
    matmul moving operand)
  - MLP runs weight-stationary ([H,512-edge] tiles, fp32 matmuls), LN runs
    in [edge, H] layout after a PE transpose, with fused
    tensor_tensor_reduce stats.
All shapes/sharding hardcoded per spec.
"""

import sys
import numpy as np

sys.path.insert(0, "/opt/trn_rl_repo")

import ml_dtypes  # noqa: E402

import concourse.bacc as bacc  # noqa: E402
import concourse.tile as tile  # noqa: E402
import concourse.mybir as mybir  # noqa: E402
from concourse.masks import make_identity  # noqa: E402

N_CORES = 8
N_ATOM = 32000
E_EDGE = 512000
D_IN = 256
H = 128
P = 128
ESH = E_EDGE // N_CORES          # 64000 edges per core
SUP = 512                        # edges per supertile (= PSUM bank)
NSUP = ESH // SUP                # 125
NPAD = 32768                     # atom table padded (ap_gather free-dim cap)
GBATCH = 1024                    # edges per ap_gather call
LN_EPS = 1e-5

F32 = mybir.dt.float32
BF16 = mybir.dt.bfloat16
I16 = mybir.dt.int16
AF = mybir.ActivationFunctionType
ALU = mybir.AluOpType

_CACHE = {}


def _build(trivial_affine: bool, nsup: int = NSUP):
    esh = nsup * SUP
    nc = bacc.Bacc("TRN2", target_bir_lowering=False, debug=False,
                   enable_asserts=False, num_devices=N_CORES)

    ef_d = nc.dram_tensor("ef", [esh, H], F32, kind="ExternalInput")
    atomT_d = nc.dram_tensor("atomT", [2, P, NPAD], BF16, kind="ExternalInput")
    idxd_d = nc.dram_tensor("idx_dst", [P, esh // 16], I16, kind="ExternalInput")
    idxs_d = nc.dram_tensor("idx_src", [P, esh // 16], I16, kind="ExternalInput")
    wlin_d = nc.dram_tensor("wlin", [D_IN, H], F32, kind="ExternalInput")
    w1_d = nc.dram_tensor("w1", [3 * H, H], F32, kind="ExternalInput")
    w2_d = nc.dram_tensor("w2", [H, H], F32, kind="ExternalInput")
    w3_d = nc.dram_tensor("w3", [H, H], F32, kind="ExternalInput")
    b1_d = nc.dram_tensor("b1", [H, 1], F32, kind="ExternalInput")
    b2_d = nc.dram_tensor("b2", [H, 1], F32, kind="ExternalInput")
    b3_d = nc.dram_tensor("b3", [H, 1], F32, kind="ExternalInput")
    if not trivial_affine:
        gam_d = nc.dram_tensor("gam", [P, H], F32, kind="ExternalInput")
        bet_d = nc.dram_tensor("bet", [P, H], F32, kind="ExternalInput")
    out_d = nc.dram_tensor("out", [esh, H], F32, kind="ExternalOutput")

    with tile.TileContext(nc) as tc:
        with tc.tile_pool(name="const", bufs=1) as const:
            # --- constants ---------------------------------------------------
            w1a = const.tile([P, H], F32)
            nc.sync.dma_start(out=w1a[:], in_=w1_d[0:H, :])
            w1b = const.tile([P, H], F32)
            nc.sync.dma_start(out=w1b[:], in_=w1_d[H:2 * H, :])
            w1c = const.tile([P, H], F32)
            nc.sync.dma_start(out=w1c[:], in_=w1_d[2 * H:3 * H, :])
            w2 = const.tile([P, H], F32)
            nc.sync.dma_start(out=w2[:], in_=w2_d[:])
            w3 = const.tile([P, H], F32)
            nc.sync.dma_start(out=w3[:], in_=w3_d[:])
            b1 = const.tile([P, 1], F32)
            nc.sync.dma_start(out=b1[:], in_=b1_d[:])
            b2 = const.tile([P, 1], F32)
            nc.sync.dma_start(out=b2[:], in_=b2_d[:])
            b3 = const.tile([P, 1], F32)
            nc.sync.dma_start(out=b3[:], in_=b3_d[:])
            if not trivial_affine:
                gam = const.tile([P, H], F32)
                nc.sync.dma_start(out=gam[:], in_=gam_d[:])
                bet = const.tile([P, H], F32)
                nc.sync.dma_start(out=bet[:], in_=bet_d[:])
            ident = const.tile([P, P], F32)
            make_identity(nc, ident[:])
            eps_t = const.tile([P, 1], F32)
            nc.vector.memset(eps_t[:], LN_EPS)
            idxd = const.tile([P, esh // 16], I16)
            nc.sync.dma_start(out=idxd[:], in_=idxd_d[:])
            idxs = const.tile([P, esh // 16], I16)
            nc.sync.dma_start(out=idxs[:], in_=idxs_d[:])
            table = const.tile([P, NPAD], F32)          # 128KB/partition

            # --- atom-scalar table build ------------------------------------
            CHUNK = 4096
            with tc.tile_pool(name="bld", bufs=2) as bld, \
                 tc.tile_pool(name="bldps", bufs=4, space="PSUM") as bldps:
                wl32a = bld.tile([P, H], F32, tag="wl32")
                nc.sync.dma_start(out=wl32a[:], in_=wlin_d[0:P, :])
                wl32b = bld.tile([P, H], F32, tag="wl32")
                nc.sync.dma_start(out=wl32b[:], in_=wlin_d[P:2 * P, :])
                wl16a = bld.tile([P, H], BF16, tag="wl16")
                nc.vector.tensor_copy(wl16a[:], wl32a[:])
                wl16b = bld.tile([P, H], BF16, tag="wl16")
                nc.vector.tensor_copy(wl16b[:], wl32b[:])
                for ci in range(NPAD // CHUNK):
                    off = ci * CHUNK
                    a0 = bld.tile([P, CHUNK], BF16, tag="a0")
                    nc.sync.dma_start(out=a0[:], in_=atomT_d[0, :, off:off + CHUNK])
                    a1 = bld.tile([P, CHUNK], BF16, tag="a1")
                    nc.sync.dma_start(out=a1[:], in_=atomT_d[1, :, off:off + CHUNK])
                    for si in range(CHUNK // SUP):
                        s = si * SUP
                        ps = bldps.tile([P, SUP], F32, space="PSUM", tag="bps")
                        nc.tensor.matmul(out=ps[:], lhsT=wl16a[:],
                                         rhs=a0[:, s:s + SUP], start=True, stop=False)
                        nc.tensor.matmul(out=ps[:], lhsT=wl16b[:],
                                         rhs=a1[:, s:s + SUP], start=False, stop=True)
                        if si % 2 == 0:
                            nc.vector.tensor_copy(table[:, off + s:off + s + SUP], ps[:])
                        else:
                            nc.scalar.copy(table[:, off + s:off + s + SUP], ps[:])

            # --- main loop ---------------------------------------------------
            SGB = GBATCH // SUP
            with tc.tile_pool(name="io", bufs=3) as io, \
                 tc.tile_pool(name="gat", bufs=2) as gat, \
                 tc.tile_pool(name="mid", bufs=2) as mid, \
                 tc.tile_pool(name="stat", bufs=3) as stat, \
                 tc.tile_pool(name="ptr", bufs=3, space="PSUM") as ptr, \
                 tc.tile_pool(name="pmm", bufs=3, space="PSUM") as pmm:
                gd = gs = None
                for t in range(nsup):
                    if t % SGB == 0:
                        gn = min(GBATCH, (nsup - t) * SUP)
                        i0 = t * (SUP // 16)
                        i1 = i0 + gn // 16
                        gd = gat.tile([P, GBATCH], F32, tag="gd")
                        nc.gpsimd.ap_gather(
                            out_ap=gd[:, :gn], in_ap=table[:], idxs_ap=idxd[:, i0:i1],
                            channels=P, num_elems=NPAD, d=1, num_idxs=gn)
                        gs = gat.tile([P, GBATCH], F32, tag="gs")
                        nc.gpsimd.ap_gather(
                            out_ap=gs[:, :gn], in_ap=table[:], idxs_ap=idxs[:, i0:i1],
                            channels=P, num_elems=NPAD, d=1, num_idxs=gn)
                    k = (t % SGB) * SUP

                    ef = io.tile([P, 4, P], F32, tag="ef")
                    nc.sync.dma_start(
                        out=ef[:],
                        in_=ef_d[t * SUP:(t + 1) * SUP, :].rearrange(
                            "(c p) f -> p c f", p=P))

                    # edge-feature transpose -> [f, e] for the L1 matmul
                    efT_ps = ptr.tile([P, 4, P], F32, space="PSUM", tag="tr")
                    for c in range(4):
                        nc.tensor.transpose(efT_ps[:, c], ef[:, c], ident[:])
                    efT = mid.tile([P, 4 * P], F32, tag="efT")
                    nc.vector.tensor_copy(efT[:], efT_ps[:].rearrange("p c f -> p (c f)"))

                    ps1 = pmm.tile([P, SUP], F32, space="PSUM", tag="mm")
                    nc.tensor.matmul(out=ps1[:], lhsT=w1a[:], rhs=gd[:, k:k + SUP],
                                     start=True, stop=False)
                    nc.tensor.matmul(out=ps1[:], lhsT=w1b[:], rhs=gs[:, k:k + SUP],
                                     start=False, stop=False)
                    nc.tensor.matmul(out=ps1[:], lhsT=w1c[:], rhs=efT[:],
                                     start=False, stop=True)
                    h1 = mid.tile([P, SUP], F32, tag="h1")
                    nc.scalar.activation(h1[:], ps1[:], AF.Relu, bias=b1[:, 0:1])

                    ps2 = pmm.tile([P, SUP], F32, space="PSUM", tag="mm")
                    nc.tensor.matmul(out=ps2[:], lhsT=w2[:], rhs=h1[:],
                                     start=True, stop=True)
                    h2 = mid.tile([P, SUP], F32, tag="h2")
                    nc.scalar.activation(h2[:], ps2[:], AF.Relu, bias=b2[:, 0:1])

                    ps3 = pmm.tile([P, SUP], F32, space="PSUM", tag="mm")
                    nc.tensor.matmul(out=ps3[:], lhsT=w3[:], rhs=h2[:],
                                     start=True, stop=True)
                    h3 = mid.tile([P, SUP], F32, tag="h3")
                    nc.scalar.activation(h3[:], ps3[:], AF.Identity, bias=b3[:, 0:1])

                    # transpose h3 back to [e, h]
                    h3T_ps = ptr.tile([P, 4, P], F32, space="PSUM", tag="tr")
                    for c in range(4):
                        nc.tensor.transpose(h3T_ps[:, c], h3[:, c * P:(c + 1) * P],
                                            ident[:])

                    # residual + LN stats (bn_stats), [e, h] layout
                    x = mid.tile([P, 4, P], F32, tag="x")
                    nc.vector.tensor_tensor(
                        out=x[:].rearrange("p c f -> p (c f)"),
                        in0=h3T_ps[:].rearrange("p c f -> p (c f)"),
                        in1=ef[:].rearrange("p c f -> p (c f)"), op=ALU.add)
                    bn = stat.tile([P, 4, 6], F32, tag="bn")
                    mv = stat.tile([P, 4, 2], F32, tag="mv")
                    for c in range(4):
                        nc.vector.bn_stats(bn[:, c], x[:, c])
                        nc.vector.bn_aggr(mv[:, c], bn[:, c])
                    mean = stat.tile([P, 4], F32, tag="mean")
                    nc.vector.tensor_copy(mean[:], mv[:, :, 0])
                    var = stat.tile([P, 4], F32, tag="var")
                    nc.vector.tensor_copy(var[:], mv[:, :, 1])
                    std = stat.tile([P, 4], F32, tag="std")
                    nc.scalar.activation(std[:], var[:], AF.Sqrt, bias=eps_t[:, 0:1])
                    rstd = stat.tile([P, 4], F32, tag="rstd")
                    nc.vector.reciprocal(rstd[:], std[:])
                    nmr = stat.tile([P, 4], F32, tag="nmr")      # -mean*rstd
                    nc.vector.tensor_tensor(out=nmr[:], in0=mean[:], in1=rstd[:],
                                            op=ALU.mult)
                    nc.vector.tensor_scalar(out=nmr[:], in0=nmr[:], scalar1=-1.0,
                                            scalar2=None, op0=ALU.mult)

                    xn = io.tile([P, 4, P], F32, tag="xn")
                    for c in range(4):
                        nc.scalar.activation(xn[:, c], x[:, c], AF.Identity,
                                             bias=nmr[:, c:c + 1],
                                             scale=rstd[:, c:c + 1])
                    if not trivial_affine:
                        for c in range(4):
                            nc.vector.tensor_tensor(out=xn[:, c], in0=xn[:, c],
                                                    in1=gam[:], op=ALU.mult)
                            nc.vector.tensor_tensor(out=xn[:, c], in0=xn[:, c],
                                                    in1=bet[:], op=ALU.add)

                    nc.sync.dma_start(
                        out=out_d[t * SUP:(t + 1) * SUP, :].rearrange(
                            "(c p) f -> p c f", p=P),
                        in_=xn[:])
    nc.compile()
    return nc


def _make_runner(nc):
    """shard_map runner over 8 cores (no donation so it can be re-invoked)."""
    import jax
    from jax.sharding import Mesh, PartitionSpec
    from jax.experimental.shard_map import shard_map
    from concourse import bass2jax

    bass2jax.install_neuronx_cc_hook()

    partition_name = (nc.partition_id_tensor.name
                      if nc.partition_id_tensor else None)
    in_names, out_names, out_avals, zero_shapes = [], [], [], []
    for alloc in nc.m.functions[0].allocations:
        if not isinstance(alloc, mybir.MemoryLocationSet):
            continue
        name = alloc.memorylocations[0].name
        if alloc.kind == "ExternalInput":
            if name != partition_name:
                in_names.append(name)
        elif alloc.kind == "ExternalOutput":
            out_names.append(name)
            out_avals.append(jax.core.ShapedArray(
                tuple(alloc.tensor_shape), mybir.dt.np(alloc.dtype)))
            zero_shapes.append((tuple(alloc.tensor_shape), mybir.dt.np(alloc.dtype)))
    n_params = len(in_names)
    all_in_names = in_names + out_names
    if partition_name is not None:
        all_in_names = all_in_names + [partition_name]

    def _body(*args):
        operands = list(args)
        if partition_name is not None:
            operands.append(bass2jax.partition_id_tensor())
        outs = bass2jax._bass_exec_p.bind(
            *operands,
            out_avals=tuple(out_avals),
            in_names=tuple(all_in_names),
            out_names=tuple(out_names),
            lowering_input_output_aliases=(),
            sim_require_finite=True,
            sim_require_nnan=True,
            nc=nc,
        )
        return tuple(outs)

    devices = jax.devices()[:N_CORES]
    mesh = Mesh(np.asarray(devices), ("core",))
    nin = n_params + len(out_names)
    sharded = jax.jit(
        shard_map(_body, mesh=mesh,
                  in_specs=(PartitionSpec("core"),) * nin,
                  out_specs=(PartitionSpec("core"),) * len(out_names),
                  check_rep=False),
        keep_unused=True)
    return sharded, in_names, out_names, zero_shapes


def _wrap_idx_n(idx_flat: np.ndarray, esh: int) -> np.ndarray:
    """ap_gather wrapped-index layout: idx[p, s] covers edge s*16 + p%16,
    replicated across the 8 gpsimd 16-partition core groups."""
    a = idx_flat.astype(np.int16).reshape(esh // 16, 16).T   # [16, esh//16]
    return np.tile(a, (8, 1))                                # [128, esh//16]


def _wrap_idx(idx_flat: np.ndarray) -> np.ndarray:
    return _wrap_idx_n(idx_flat, ESH)


def _prep(inputs):
    atom_features = np.asarray(inputs["atom_features"], dtype=np.float32)
    edge_features = np.asarray(inputs["edge_features"], dtype=np.float32)
    edge_index = np.asarray(inputs["edge_index"]).astype(np.int64)
    wlin = np.asarray(inputs["W_lin"], dtype=np.float32)
    w1 = np.asarray(inputs["W1"], dtype=np.float32)
    w2 = np.asarray(inputs["W2"], dtype=np.float32)
    w3 = np.asarray(inputs["W3"], dtype=np.float32)
    b1 = np.asarray(inputs["b1"], dtype=np.float32).reshape(H, 1)
    b2 = np.asarray(inputs["b2"], dtype=np.float32).reshape(H, 1)
    b3 = np.asarray(inputs["b3"], dtype=np.float32).reshape(H, 1)
    gamma = np.asarray(inputs["gamma"], dtype=np.float32)
    beta = np.asarray(inputs["beta"], dtype=np.float32)

    trivial_affine = bool(np.all(gamma == 1.0) and np.all(beta == 0.0))

    atomT = np.zeros((2, P, NPAD), dtype=ml_dtypes.bfloat16)
    at = atom_features.T.astype(ml_dtypes.bfloat16)          # [256, 32000]
    atomT[0, :, :N_ATOM] = at[:P]
    atomT[1, :, :N_ATOM] = at[P:]

    shared = {
        "atomT": atomT, "wlin": wlin, "w1": w1, "w2": w2, "w3": w3,
        "b1": b1, "b2": b2, "b3": b3,
    }
    if not trivial_affine:
        shared["gam"] = np.tile(gamma.reshape(1, H), (P, 1)).astype(np.float32)
        shared["bet"] = np.tile(beta.reshape(1, H), (P, 1)).astype(np.float32)

    in_maps = []
    for c in range(N_CORES):
        e0 = c * ESH
        m = dict(shared)
        m["ef"] = edge_features[e0:e0 + ESH]
        m["idx_dst"] = _wrap_idx(edge_index[0, e0:e0 + ESH])
        m["idx_src"] = _wrap_idx(edge_index[1, e0:e0 + ESH])
        in_maps.append(m)
    return in_maps, trivial_affine


def _get_compiled(trivial_affine: bool):
    key = ("k", trivial_affine)
    if key not in _CACHE:
        nc = _build(trivial_affine)
        runner = _make_runner(nc)
        _CACHE[key] = (nc, runner)
    return _CACHE[key]


def _concat_inputs(in_maps, in_names, zero_shapes):
    concat_in = [
        np.concatenate([np.asarray(in_maps[c][n]) for c in range(N_CORES)], axis=0)
        for n in in_names
    ]
    concat_zero = [
        np.zeros((N_CORES * s[0], *s[1:]), dt) for (s, dt) in zero_shapes
    ]
    return concat_in, concat_zero


def kernel(**inputs) -> np.ndarray:
    in_maps, trivial_affine = _prep(inputs)
    _, (sharded, in_names, out_names, zero_shapes) = _get_compiled(trivial_affine)
    concat_in, concat_zero = _concat_inputs(in_maps, in_names, zero_shapes)
    outs = sharded(*concat_in, *concat_zero)
    oi = out_names.index("out")
    full = np.asarray(outs[oi]).reshape(N_CORES * ESH, H)
    return full.astype(np.float32)


def bench(inputs, reps: int = 10):
    """Returns (exec_times_seconds, results) using device-resident inputs."""
    import jax, time
    in_maps, trivial_affine = _prep(inputs)
    _, (sharded, in_names, out_names, zero_shapes) = _get_compiled(trivial_affine)
    concat_in, concat_zero = _concat_inputs(in_maps, in_names, zero_shapes)
    args = [jax.device_put(a) for a in concat_in + concat_zero]
    outs = sharded(*args)  # warm-up + compile
    jax.block_until_ready(outs)
    times = []
    for _ in range(reps):
        t0 = time.perf_counter()
        outs = sharded(*args)
        jax.block_until_ready(outs)
        times.append(time.perf_counter() - t0)
    oi = out_names.index("out")
    full = np.asarray(outs[oi]).reshape(N_CORES * ESH, H).astype(np.float32)
    return times, full
